# revision 1
# baseline (speedup 1.0000x reference)
"""Trainium2 Bass kernel for nn_Attention_16655883174794.

FiLM-modulated 16-neighbor attention + LayerNorm + ReLU + out-proj + max-pool
over the neighbor axis. Data-parallel over bs=8 across 8 NeuronCores.

Key structural choices:
 - Host marshals inputs to feature-major (transposed) bf16: xT [128, ntok],
   ctxT [7, ntok]. The FiLM additive path (cb) is folded into the q/k/v
   projections via precomputed W2* = Wcb @ W* and fused biases.
 - Attention over groups of 16 runs as block-diagonal 128x128 PE matmuls
   (8 groups per "pack"), with the off-diagonal garbage killed by a -65536
   additive mask realized as a rank-8 PE matmul (Amask^T @ Bmask).
 - Softmax is UNNORMALIZED (no max-subtract: logits are small; no rowsum:
   LayerNorm is scale-invariant per token, so 1/rowsum cancels).
 - v is produced token-major (ckx slice as the stationary operand) so the
   attention-weighted sum lands token-major for the per-token LayerNorm.
 - LN stats per token on DVE/ACT (rsqrt via Ln+Exp, same ACT table set as
   the softmax Exp); out-proj needs feature-major, done via PE transpose.
 - max over the 16 neighbors = grouped free-dim reduce in feature-major.

Self-contained: hardcodes shapes bs=8, pn=4096, k=16, d=128.
"""
import sys
sys.path.insert(0, '/opt/trn_rl_repo')

import numpy as np
import ml_dtypes
from contextlib import ExitStack

from concourse import bacc, mybir
import concourse.tile as tile
from concourse.bass_utils import run_bass_kernel_spmd
from concourse.masks import make_identity

F32 = mybir.dt.float32
BF16 = mybir.dt.bfloat16
BF = ml_dtypes.bfloat16

B, PN, KN, D = 8, 4096, 16, 128        # bs, point_num, neighbors, dim
CTX = 7
SCALE = 1.0 / float(np.sqrt(D))
TT = 512                                # tokens per tile (4 packs of 128)
CHT = 8192                              # ctx tokens per resident chunk

last_exec_time_ns = None
_cache = {}
STAGE = 99   # debug bisect: 1=ck/FiLM 2=+qk 3=+v 4=+S/exp 5=+AV/LN 6=full


def _build(ntok, use_g, use_b):
    """Build the per-core program for ntok tokens (= pn_shard * 16)."""
    ntiles = ntok // TT
    npts = ntok // KN
    cht = min(CHT, ntok)
    tiles_per_chunk = cht // TT
    npk = TT // D                       # packs per tile (4)

    nc = bacc.Bacc()
    xT = nc.declare_dram_parameter("xT", [D, ntok], BF16, isOutput=False)
    cT = nc.declare_dram_parameter("cT", [CTX, ntok], BF16, isOutput=False)
    Wck = nc.declare_dram_parameter("Wck", [CTX, D], BF16, isOutput=False)
    Wq = nc.declare_dram_parameter("Wq", [D, D], BF16, isOutput=False)
    Wk = nc.declare_dram_parameter("Wk", [D, D], BF16, isOutput=False)
    Wv = nc.declare_dram_parameter("Wv", [D, D], BF16, isOutput=False)
    W2q = nc.declare_dram_parameter("W2q", [CTX, D], BF16, isOutput=False)
    W2k = nc.declare_dram_parameter("W2k", [CTX, D], BF16, isOutput=False)
    W2v = nc.declare_dram_parameter("W2v", [CTX, D], BF16, isOutput=False)
    Wo = nc.declare_dram_parameter("Wo", [D, D], BF16, isOutput=False)
    Am = nc.declare_dram_parameter("Am", [8, D], BF16, isOutput=False)
    Bm = nc.declare_dram_parameter("Bm", [8, D], BF16, isOutput=False)
    bck = nc.declare_dram_parameter("bck", [D, 1], F32, isOutput=False)
    bq = nc.declare_dram_parameter("bq", [D, 1], F32, isOutput=False)
    bk = nc.declare_dram_parameter("bk", [D, 1], F32, isOutput=False)
    bvr = nc.declare_dram_parameter("bvr", [1, D], BF16, isOutput=False)
    bo = nc.declare_dram_parameter("bo", [D, 1], F32, isOutput=False)
    gb = nc.declare_dram_parameter("gb", [D, 2 * D], F32, isOutput=False)
    outT = nc.declare_dram_parameter("outT", [D, npts], F32, isOutput=True)

    with ExitStack() as ctx:
        tc = ctx.enter_context(tile.TileContext(nc))
        wp = ctx.enter_context(tc.tile_pool(name="wp", bufs=1))
        cp = ctx.enter_context(tc.tile_pool(name="cp", bufs=2))
        xp = ctx.enter_context(tc.tile_pool(name="xp", bufs=3))
        mp = ctx.enter_context(tc.tile_pool(name="mp", bufs=2))
        sp = ctx.enter_context(tc.tile_pool(name="sp", bufs=2))
        avp = ctx.enter_context(tc.tile_pool(name="avp", bufs=2 * npk + 1))
        og = ctx.enter_context(tc.tile_pool(name="og", bufs=1))
        bigps = ctx.enter_context(tc.tile_pool(name="bigps", bufs=3, space="PSUM"))
        pkps = ctx.enter_context(tc.tile_pool(name="pkps", bufs=3, space="PSUM"))
        tpps = ctx.enter_context(tc.tile_pool(name="tpps", bufs=2, space="PSUM"))

        # ---- persistent constants ----
        wck_sb = wp.tile([CTX, D], BF16, name="wck_sb")
        wq_sb = wp.tile([D, D], BF16, name="wq_sb")
        wk_sb = wp.tile([D, D], BF16, name="wk_sb")
        wv_sb = wp.tile([D, D], BF16, name="wv_sb")
        w2q_sb = wp.tile([CTX, D], BF16, name="w2q_sb")
        w2k_sb = wp.tile([CTX, D], BF16, name="w2k_sb")
        w2v_sb = wp.tile([CTX, D], BF16, name="w2v_sb")
        wo_sb = wp.tile([D, D], BF16, name="wo_sb")
        am_sb = wp.tile([8, D], BF16, name="am_sb")
        bm_sb = wp.tile([8, D], BF16, name="bm_sb")
        bck_sb = wp.tile([D, 1], F32, name="bck_sb")
        bq_sb = wp.tile([D, 1], F32, name="bq_sb")
        bk_sb = wp.tile([D, 1], F32, name="bk_sb")
        bvr_sb = wp.tile([1, D], BF16, name="bvr_sb")
        bo_sb = wp.tile([D, 1], F32, name="bo_sb")
        gb_sb = wp.tile([D, 2 * D], F32, name="gb_sb")
        ident = wp.tile([D, D], BF16, name="ident")
        ones_col = wp.tile([1, D], BF16, name="ones_col")
        for dst, src in [(wck_sb, Wck), (wq_sb, Wq), (wk_sb, Wk), (wv_sb, Wv),
                         (w2q_sb, W2q), (w2k_sb, W2k), (w2v_sb, W2v),
                         (wo_sb, Wo), (am_sb, Am), (bm_sb, Bm), (bck_sb, bck),
                         (bq_sb, bq), (bk_sb, bk), (bvr_sb, bvr), (bo_sb, bo),
                         (gb_sb, gb)]:
            nc.sync.dma_start(out=dst, in_=src[:])
        make_identity(nc, ident)
        nc.vector.memset(ones_col, 1.0)

        stage = og.tile([D, npts], F32, name="stage")
        ctx_ch = None

        for t in range(ntiles):
            if t % tiles_per_chunk == 0:
                ctx_ch = cp.tile([CTX, cht], BF16, name="ctx_ch", tag="ctx_ch")
                nc.sync.dma_start(out=ctx_ch, in_=cT[:, t * TT:t * TT + cht])
            coff = (t % tiles_per_chunk) * TT
            ctx_t = ctx_ch[:, coff:coff + TT]

            x_t = xp.tile([D, TT], BF16, name="x_t", tag="x_t")
            nc.sync.dma_start(out=x_t, in_=xT[:, t * TT:(t + 1) * TT])

            # ck = Wck^T @ ctx  (feature-major [D, TT]),  + bck on eviction
            ck_ps = bigps.tile([D, TT], F32, name="ck_ps", tag="big")
            nc.tensor.matmul(ck_ps, wck_sb, ctx_t, start=True, stop=True)
            # fused FiLM: ckx = (ck + bck) * x in one DVE pass from PSUM
            ckx = mp.tile([D, TT], BF16, name="ckx", tag="ckx")
            nc.vector.scalar_tensor_tensor(ckx, ck_ps, bck_sb, x_t,
                                           op0=mybir.AluOpType.add,
                                           op1=mybir.AluOpType.mult)

            if STAGE < 2:
                nc.vector.tensor_reduce(
                    stage[:, t * (TT // KN):(t + 1) * (TT // KN)],
                    ckx.rearrange("p (g k) -> p g k", k=KN),
                    axis=mybir.AxisListType.X, op=mybir.AluOpType.max)
                continue
            # q/k projections, feature-major; cb-path via W2*, bias on evict
            q_ps = bigps.tile([D, TT], F32, name="q_ps", tag="big")
            nc.tensor.matmul(q_ps, wq_sb, ckx, start=True, stop=False)
            nc.tensor.matmul(q_ps, w2q_sb, ctx_t, start=False, stop=True)
            q_sb = mp.tile([D, TT], BF16, name="q_sb", tag="q_sb")
            nc.scalar.activation(q_sb, q_ps,
                                 mybir.ActivationFunctionType.Identity,
                                 bias=bq_sb, scale=1.0)

            k_ps = bigps.tile([D, TT], F32, name="k_ps", tag="big")
            nc.tensor.matmul(k_ps, wk_sb, ckx, start=True, stop=False)
            nc.tensor.matmul(k_ps, w2k_sb, ctx_t, start=False, stop=True)
            k_sb = mp.tile([D, TT], BF16, name="k_sb", tag="k_sb")
            nc.scalar.activation(k_sb, k_ps,
                                 mybir.ActivationFunctionType.Identity,
                                 bias=bk_sb, scale=1.0)

            if STAGE < 3:
                nc.vector.tensor_reduce(
                    stage[:, t * (TT // KN):(t + 1) * (TT // KN)],
                    q_sb.rearrange("p (g k) -> p g k", k=KN),
                    axis=mybir.AxisListType.X, op=mybir.AluOpType.max)
                continue
            # v projection, TOKEN-major: v[j,e] = ckx[:,j]^T Wv + ctx[:,j]^T W2v + bv
            v_ps = bigps.tile([D, TT], F32, name="v_ps", tag="big")
            for p in range(npk):
                sl = slice(p * D, (p + 1) * D)
                nc.tensor.matmul(v_ps[:, sl], ckx[:, sl], wv_sb,
                                 start=True, stop=False)
                nc.tensor.matmul(v_ps[:, sl], ctx_t[:, sl], w2v_sb,
                                 start=False, stop=False)
                nc.tensor.matmul(v_ps[:, sl], ones_col, bvr_sb,
                                 start=False, stop=True)
            v_sb = mp.tile([D, TT], BF16, name="v_sb", tag="v_sb")
            nc.vector.tensor_copy(v_sb, v_ps)

            if STAGE < 4:
                nc.vector.tensor_reduce(
                    stage[:, t * (TT // KN):(t + 1) * (TT // KN)],
                    v_sb.rearrange("p (g k) -> p g k", k=KN),
                    axis=mybir.AxisListType.X, op=mybir.AluOpType.max)
                continue
            avs = sp.tile([D, npk], F32, name="avs", tag="avs")
            sqs = sp.tile([D, npk], F32, name="sqs", tag="sqs")
            av_tiles = []

            tT_sb_dbg = mp.tile([D, TT], BF16, name="tT_sb_dbg", tag="tT_sb") if STAGE < 5.5 else None
            for p in range(npk):
                sl = slice(p * D, (p + 1) * D)
                # S^T[j,i] = k_j . q_i  + block-diagonal -65536 mask
                st_ps = pkps.tile([D, D], F32, name="st_ps", tag="pk")
                nc.tensor.matmul(st_ps, k_sb[:, sl], q_sb[:, sl],
                                 start=True, stop=False)
                nc.tensor.matmul(st_ps, am_sb, bm_sb, start=False, stop=True)
                et_sb = sp.tile([D, D], BF16, name="et_sb", tag="et_sb")
                nc.scalar.activation(et_sb, st_ps,
                                     mybir.ActivationFunctionType.Exp,
                                     scale=SCALE)
                if STAGE < 5:
                    nc.vector.tensor_copy(tT_sb_dbg[:, sl], et_sb)
                    continue
                # av[i,e] = sum_j et[j,i] v[j,e]   (token-major, unnormalized)
                av_ps = pkps.tile([D, D], F32, name="av_ps", tag="pk")
                nc.tensor.matmul(av_ps, et_sb, v_sb[:, sl],
                                 start=True, stop=True)
                av_sb = avp.tile([D, D], F32, name="av_sb", tag="av_sb")
                if STAGE < 5.2:
                    nc.scalar.activation(av_sb, av_ps,
                                         mybir.ActivationFunctionType.Identity,
                                         bias=0.0, scale=1.0)
                    nc.vector.tensor_copy(tT_sb_dbg[:, sl], av_sb)
                    continue
                nc.scalar.activation(av_sb, av_ps,
                                     mybir.ActivationFunctionType.Identity,
                                     bias=0.0, scale=1.0,
                                     accum_out=avs[:, p:p + 1])
                if STAGE < 5.4:
                    nc.vector.tensor_copy(tT_sb_dbg[:, sl], av_sb)
                    continue
                sq_sc = sp.tile([D, D], F32, name="sq_sc", tag="sq_sc")
                nc.scalar.activation(sq_sc, av_sb,
                                     mybir.ActivationFunctionType.Square,
                                     accum_out=sqs[:, p:p + 1])
                av_tiles.append(av_sb)

            if STAGE < 5.5:
                nc.vector.tensor_reduce(
                    stage[:, t * (TT // KN):(t + 1) * (TT // KN)],
                    tT_sb_dbg.rearrange("p (g k) -> p g k", k=KN),
                    axis=mybir.AxisListType.X, op=mybir.AluOpType.max)
                continue
            # batched LN stats: -mean, variance, rsigma = exp(-0.5 ln(var+eps))
            negmu = sp.tile([D, npk], F32, name="negmu", tag="negmu")
            nc.vector.tensor_scalar_mul(negmu, avs, -1.0 / D)
            var = sp.tile([D, npk], F32, name="var", tag="var")
            nc.vector.tensor_scalar(var, sqs, 1.0 / D, 1e-5,
                                    op0=mybir.AluOpType.mult,
                                    op1=mybir.AluOpType.add)
            musq = sp.tile([D, npk], F32, name="musq", tag="musq")
            nc.vector.tensor_mul(musq, negmu, negmu)
            nc.vector.tensor_sub(var, var, musq)
            lnv = sp.tile([D, npk], F32, name="lnv", tag="lnv")
            nc.scalar.activation(lnv, var, mybir.ActivationFunctionType.Ln,
                                 bias=0.0, scale=1.0)
            rsig = sp.tile([D, npk], F32, name="rsig", tag="rsig")
            nc.scalar.activation(rsig, lnv, mybir.ActivationFunctionType.Exp,
                                 scale=-0.5)

            if STAGE < 6:
                nc.vector.tensor_reduce(
                    stage[:, t * (TT // KN):(t + 1) * (TT // KN)],
                    av_tiles[0].rearrange("p (g k) -> p g k", k=KN),
                    axis=mybir.AxisListType.X, op=mybir.AluOpType.max)
                continue
            tT_sb = mp.tile([D, TT], BF16, name="tT_sb", tag="tT_sb")
            for p in range(npk):
                sl = slice(p * D, (p + 1) * D)
                av_sb = av_tiles[p]
                # z = (av - mu) * rsigma  (per-token scalars on partitions)
                z = sp.tile([D, D], F32, name="z", tag="z")
                nc.vector.tensor_scalar(z, av_sb, negmu[:, p:p + 1],
                                        rsig[:, p:p + 1],
                                        op0=mybir.AluOpType.add,
                                        op1=mybir.AluOpType.mult)
                if use_g:
                    nc.vector.tensor_mul(z, z, gb_sb[:, 0:D])
                if use_b:
                    nc.vector.tensor_add(z, z, gb_sb[:, D:2 * D])
                t_sb = sp.tile([D, D], BF16, name="t_sb", tag="t_sb")
                nc.vector.tensor_scalar_max(t_sb, z, 0.0)
                # transpose to feature-major for the out-projection
                tT_ps = tpps.tile([D, D], BF16, name="tT_ps", tag="tp")
                nc.tensor.transpose(tT_ps, t_sb, ident)
                nc.vector.tensor_copy(tT_sb[:, sl], tT_ps)

            # out-projection (feature-major) + max over the 16 neighbors
            oT_ps = bigps.tile([D, TT], F32, name="oT_ps", tag="big")
            nc.tensor.matmul(oT_ps, wo_sb, tT_sb, start=True, stop=True)
            nc.vector.tensor_reduce(
                stage[:, t * (TT // KN):(t + 1) * (TT // KN)],
                oT_ps.rearrange("p (g k) -> p g k", k=KN),
                axis=mybir.AxisListType.X, op=mybir.AluOpType.max)

        nc.vector.tensor_scalar_add(stage, stage, bo_sb)
        nc.sync.dma_start(out=outT[:], in_=stage)

    nc.compile()
    return nc


def kernel(x, context, Wck, bck, Wcb, bcb, Wq, bq, Wk, bk, Wv, bv,
           ln_g, ln_b, Wo, bo):
    """Full-input entry point: shards bs across 8 cores, returns full output."""
    global last_exec_time_ns
    x = np.asarray(x, dtype=np.float32)
    context = np.asarray(context, dtype=np.float32)
    f32 = lambda a: np.asarray(a, dtype=np.float32)
    Wck, bck, Wcb, bcb = f32(Wck), f32(bck), f32(Wcb), f32(bcb)
    Wq, bq, Wk, bk, Wv, bv = f32(Wq), f32(bq), f32(Wk), f32(bk), f32(Wv), f32(bv)
    ln_g, ln_b, Wo, bo = f32(ln_g), f32(ln_b), f32(Wo), f32(bo)

    bs, pn, kn, d = x.shape
    ntok = pn * kn
    use_g = not np.allclose(ln_g, 1.0)
    use_b = np.any(ln_b != 0.0)

    key = (ntok, use_g, use_b)
    if key not in _cache:
        _cache[key] = _build(ntok, use_g, use_b)
    nc = _cache[key]

    # fold the FiLM additive path (cb = ctx@Wcb + bcb) through q/k/v
    W2q, W2k, W2v = Wcb @ Wq, Wcb @ Wk, Wcb @ Wv
    bias_q = (bq + bcb @ Wq).reshape(D, 1)
    bias_k = (bk + bcb @ Wk).reshape(D, 1)
    bias_v_row = (bv + bcb @ Wv).reshape(1, D)

    gidx = np.arange(D) // 16
    Am = (gidx[None, :] == np.arange(8)[:, None]).astype(np.float32)
    Bm = np.where(Am > 0, 0.0, -65536.0).astype(np.float32)

    bf = lambda a: np.ascontiguousarray(a, dtype=None).astype(BF)
    weights = {
        "Wck": bf(Wck), "Wq": bf(Wq), "Wk": bf(Wk), "Wv": bf(Wv),
        "W2q": bf(W2q), "W2k": bf(W2k), "W2v": bf(W2v), "Wo": bf(Wo),
        "Am": bf(Am), "Bm": bf(Bm),
        "bck": bck.reshape(D, 1), "bq": bias_q, "bk": bias_k,
        "bvr": bf(bias_v_row), "bo": bo.reshape(D, 1),
        "gb": np.concatenate([np.broadcast_to(ln_g, (D, D)),
                              np.broadcast_to(ln_b, (D, D))],
                             axis=1).astype(np.float32),
    }

    in_maps = []
    for c in range(bs):
        xc = x[c].reshape(ntok, d).T
        cc = context[c].reshape(ntok, CTX).T
        m = dict(weights)
        m["xT"] = np.ascontiguousarray(xc).astype(BF)
        m["cT"] = np.ascontiguousarray(cc).astype(BF)
        in_maps.append(m)

    import time
    t0 = time.perf_counter()
    res = run_bass_kernel_spmd(nc, in_maps, list(range(bs)))
    wall_ns = (time.perf_counter() - t0) * 1e9
    last_exec_time_ns = res.exec_time_ns if res.exec_time_ns else int(wall_ns)
    out = np.stack([res.results[c]["outT"].T for c in range(bs)])  # [B, pn, D]
    return out.astype(np.float32)



# revision 10
# speedup vs baseline: 3.1804x; 3.1804x over previous
"""Trainium2 Bass kernel for nn_Attention_16655883174794.

FiLM-modulated 16-neighbor attention + LayerNorm + ReLU + out-proj + max-pool
over the neighbor axis. Data-parallel over bs=8 across 8 NeuronCores.

Wall-clock here is dominated by the axon tunnel (~90 MB/s H2D, ~15 MB/s D2H,
~0.1-0.35 s fixed cost per transfer op), so the host/transport path is
engineered as hard as the device program:
 - x ships token-major bf16 (one contiguous astype, no host transpose; the
   PE transposes tiles on device), ctx ships feature-major bf16 (small).
 - All 16 weight/bias tensors are packed into ONE [128, CW] f32 operand.
 - The PJRT executable is jitted once and cached; output donation buffers
   are created on-device (the stock path shipped 16.8 MB of host zeros).
 - Output returns token-major f16 (half the D2H bytes, contiguous cast on
   the host side).

Device program (per core, ntok = 65536 tokens = 4096 groups of 16):
 - FiLM additive path (cb) folded into q/k/v: W2* = Wcb @ W*, fused biases.
 - Attention over groups of 16 as block-diagonal 128x128 PE matmuls with a
   rank-8 additive -65536 mask killing the off-diagonal blocks.
 - Softmax is UNNORMALIZED (no max-subtract: logits are small; no rowsum:
   LayerNorm is scale-invariant per token, so 1/rowsum cancels).
 - LN stats per token via ACT accum_out; rsqrt via Ln+Exp.
 - max over the 16 neighbors = grouped free-dim reduce in feature-major,
   then PE transposes the result back to token-major f16 for the wire.

Self-contained: hardcodes shapes bs=8, pn=4096, k=16, d=128.
"""
import sys
sys.path.insert(0, '/opt/trn_rl_repo')

import time
import numpy as np
import ml_dtypes
from contextlib import ExitStack

from concourse import bacc, mybir
import concourse.tile as tile
from concourse.masks import make_identity

F32 = mybir.dt.float32
F16 = mybir.dt.float16
BF16 = mybir.dt.bfloat16
BF = ml_dtypes.bfloat16

B, PN, KN, D = 8, 4096, 16, 128        # bs, point_num, neighbors, dim
CTX = 7
SCALE = 1.0 / float(np.sqrt(D))
TT = 512                                # tokens per tile (4 packs of 128)
CHT = 8192                              # ctx tokens per resident chunk

# column layout of the packed weights operand wf [D, CW] f32
_C_BCK, _C_BQ, _C_BK, _C_BO = 0, 1, 2, 3
_C_WCK = 4
_C_WQ = _C_WCK + D
_C_WK = _C_WQ + D
_C_WV = _C_WK + D
_C_WO = _C_WV + D
_C_W2Q = _C_WO + D
_C_W2K = _C_W2Q + D
_C_W2V = _C_W2K + D
_C_AM = _C_W2V + D
_C_BM = _C_AM + D
_C_BVR = _C_BM + D
_C_GB = _C_BVR + D
CW_BASE = _C_GB                         # 1412
last_exec_time_ns = None
_cache = {}


def _build(ntok, use_g, use_b):
    """Build the per-core program for ntok tokens (= pn_shard * 16)."""
    ntiles = ntok // TT
    npts = ntok // KN
    npk = TT // D                       # packs per tile (4)
    cw = CW_BASE + (2 * D if (use_g or use_b) else 0)

    nc = bacc.Bacc()
    xTok = nc.declare_dram_parameter("xTok", [ntok, D], BF16, isOutput=False)
    cTok = nc.declare_dram_parameter("cTok", [ntok, CTX], BF16, isOutput=False)
    wf = nc.declare_dram_parameter("wf", [D, cw], F32, isOutput=False)
    outT = nc.declare_dram_parameter("outT", [npts, D], F16, isOutput=True)

    with ExitStack() as ctx:
        tc = ctx.enter_context(tile.TileContext(nc))
        wp = ctx.enter_context(tc.tile_pool(name="wp", bufs=1))
        cp = ctx.enter_context(tc.tile_pool(name="cp", bufs=2))
        xp = ctx.enter_context(tc.tile_pool(name="xp", bufs=3))
        mp = ctx.enter_context(tc.tile_pool(name="mp", bufs=2))
        sp = ctx.enter_context(tc.tile_pool(name="sp", bufs=2))
        avp = ctx.enter_context(tc.tile_pool(name="avp", bufs=2 * npk + 1))
        og = ctx.enter_context(tc.tile_pool(name="og", bufs=1))
        bigps = ctx.enter_context(tc.tile_pool(name="bigps", bufs=3, space="PSUM"))
        pkps = ctx.enter_context(tc.tile_pool(name="pkps", bufs=3, space="PSUM"))
        tpps = ctx.enter_context(tc.tile_pool(name="tpps", bufs=2, space="PSUM"))

        # ---- persistent constants: one DMA + on-chip bf16 extraction ----
        wf_sb = wp.tile([D, cw], F32, name="wf_sb")
        nc.sync.dma_start(out=wf_sb, in_=wf[:])
        wck_sb = wp.tile([CTX, D], BF16, name="wck_sb")
        wq_sb = wp.tile([D, D], BF16, name="wq_sb")
        wk_sb = wp.tile([D, D], BF16, name="wk_sb")
        wv_sb = wp.tile([D, D], BF16, name="wv_sb")
        wo_sb = wp.tile([D, D], BF16, name="wo_sb")
        w2q_sb = wp.tile([CTX, D], BF16, name="w2q_sb")
        w2k_sb = wp.tile([CTX, D], BF16, name="w2k_sb")
        w2v_sb = wp.tile([CTX, D], BF16, name="w2v_sb")
        am_sb = wp.tile([8, D], BF16, name="am_sb")
        bm_sb = wp.tile([8, D], BF16, name="bm_sb")
        bvr_sb = wp.tile([1, D], BF16, name="bvr_sb")
        for dst, col, rows in [(wck_sb, _C_WCK, CTX), (wq_sb, _C_WQ, D),
                               (wk_sb, _C_WK, D), (wv_sb, _C_WV, D),
                               (wo_sb, _C_WO, D), (w2q_sb, _C_W2Q, CTX),
                               (w2k_sb, _C_W2K, CTX), (w2v_sb, _C_W2V, CTX),
                               (am_sb, _C_AM, 8), (bm_sb, _C_BM, 8),
                               (bvr_sb, _C_BVR, 1)]:
            nc.vector.tensor_copy(dst, wf_sb[0:rows, col:col + D])
        bck_sb = wf_sb[:, _C_BCK:_C_BCK + 1]
        bq_sb = wf_sb[:, _C_BQ:_C_BQ + 1]
        bk_sb = wf_sb[:, _C_BK:_C_BK + 1]
        bo_sb = wf_sb[:, _C_BO:_C_BO + 1]
        gb_sb = wf_sb[:, _C_GB:_C_GB + 2 * D] if (use_g or use_b) else None
        ident = wp.tile([D, D], BF16, name="ident")
        ones_col = wp.tile([1, D], BF16, name="ones_col")
        make_identity(nc, ident)
        nc.vector.memset(ones_col, 1.0)

        stage = og.tile([D, npts], F32, name="stage")

        for t in range(ntiles):
            # token-major loads; PE transposes to feature-major on chip.
            # xa block p holds tokens [t*TT+p*128, +128) as [token, feat].
            xa = xp.tile([D, TT], BF16, name="xa", tag="xa")
            nc.sync.dma_start(
                out=xa.rearrange("a (p d) -> a p d", p=npk),
                in_=xTok[t * TT:(t + 1) * TT, :].rearrange("(p a) d -> a p d", p=npk))
            ca = cp.tile([D, npk * CTX], BF16, name="ca", tag="ca")
            nc.sync.dma_start(
                out=ca.rearrange("a (p c) -> a p c", p=npk),
                in_=cTok[t * TT:(t + 1) * TT, :].rearrange("(p a) c -> a p c", p=npk))
            x_t = xp.tile([D, TT], BF16, name="x_t", tag="x_t")
            ctx_t = cp.tile([CTX, TT], BF16, name="ctx_t", tag="ctx_t")
            for p in range(npk):
                sl = slice(p * D, (p + 1) * D)
                xt_ps = tpps.tile([D, D], BF16, name="xt_ps", tag="tp")
                nc.tensor.transpose(xt_ps, xa[:, sl], ident)
                nc.vector.tensor_copy(x_t[:, sl], xt_ps)
                ct_ps = tpps.tile([CTX, D], BF16, name="ct_ps", tag="tp")
                nc.tensor.transpose(ct_ps, ca[:, p * CTX:(p + 1) * CTX], ident)
                nc.vector.tensor_copy(ctx_t[:, sl], ct_ps)

            # ck = Wck^T @ ctx  (feature-major [D, TT]),  + bck on eviction
            ck_ps = bigps.tile([D, TT], F32, name="ck_ps", tag="big")
            nc.tensor.matmul(ck_ps, wck_sb, ctx_t, start=True, stop=True)
            # fused FiLM: ckx = (ck + bck) * x in one DVE pass from PSUM
            ckx = mp.tile([D, TT], BF16, name="ckx", tag="ckx")
            nc.vector.scalar_tensor_tensor(ckx, ck_ps, bck_sb, x_t,
                                           op0=mybir.AluOpType.add,
                                           op1=mybir.AluOpType.mult)

            # q/k projections, feature-major; cb-path via W2*, bias on evict
            q_ps = bigps.tile([D, TT], F32, name="q_ps", tag="big")
            nc.tensor.matmul(q_ps, wq_sb, ckx, start=True, stop=False)
            nc.tensor.matmul(q_ps, w2q_sb, ctx_t, start=False, stop=True)
            q_sb = mp.tile([D, TT], BF16, name="q_sb", tag="q_sb")
            nc.scalar.activation(q_sb, q_ps,
                                 mybir.ActivationFunctionType.Identity,
                                 bias=bq_sb, scale=1.0)

            k_ps = bigps.tile([D, TT], F32, name="k_ps", tag="big")
            nc.tensor.matmul(k_ps, wk_sb, ckx, start=True, stop=False)
            nc.tensor.matmul(k_ps, w2k_sb, ctx_t, start=False, stop=True)
            k_sb = mp.tile([D, TT], BF16, name="k_sb", tag="k_sb")
            nc.scalar.activation(k_sb, k_ps,
                                 mybir.ActivationFunctionType.Identity,
                                 bias=bk_sb, scale=1.0)

            # v projection, TOKEN-major: v[j,e] = ckx[:,j]^T Wv + ctx[:,j]^T W2v + bv
            v_ps = bigps.tile([D, TT], F32, name="v_ps", tag="big")
            for p in range(npk):
                sl = slice(p * D, (p + 1) * D)
                nc.tensor.matmul(v_ps[:, sl], ckx[:, sl], wv_sb,
                                 start=True, stop=False)
                nc.tensor.matmul(v_ps[:, sl], ctx_t[:, sl], w2v_sb,
                                 start=False, stop=False)
                nc.tensor.matmul(v_ps[:, sl], ones_col, bvr_sb,
                                 start=False, stop=True)
            v_sb = mp.tile([D, TT], BF16, name="v_sb", tag="v_sb")
            nc.vector.tensor_copy(v_sb, v_ps)

            avs = sp.tile([D, npk], F32, name="avs", tag="avs")
            sqs = sp.tile([D, npk], F32, name="sqs", tag="sqs")
            av_tiles = []

            for p in range(npk):
                sl = slice(p * D, (p + 1) * D)
                # S^T[j,i] = k_j . q_i  + block-diagonal -65536 mask
                st_ps = pkps.tile([D, D], F32, name="st_ps", tag="pk")
                nc.tensor.matmul(st_ps, k_sb[:, sl], q_sb[:, sl],
                                 start=True, stop=False)
                nc.tensor.matmul(st_ps, am_sb, bm_sb, start=False, stop=True)
                et_sb = sp.tile([D, D], BF16, name="et_sb", tag="et_sb")
                nc.scalar.activation(et_sb, st_ps,
                                     mybir.ActivationFunctionType.Exp,
                                     scale=SCALE)
                # av[i,e] = sum_j et[j,i] v[j,e]   (token-major, unnormalized)
                av_ps = pkps.tile([D, D], F32, name="av_ps", tag="pk")
                nc.tensor.matmul(av_ps, et_sb, v_sb[:, sl],
                                 start=True, stop=True)
                av_sb = avp.tile([D, D], F32, name="av_sb", tag="av_sb")
                nc.scalar.activation(av_sb, av_ps,
                                     mybir.ActivationFunctionType.Identity,
                                     bias=0.0, scale=1.0,
                                     accum_out=avs[:, p:p + 1])
                sq_sc = sp.tile([D, D], F32, name="sq_sc", tag="sq_sc")
                nc.scalar.activation(sq_sc, av_sb,
                                     mybir.ActivationFunctionType.Square,
                                     accum_out=sqs[:, p:p + 1])
                av_tiles.append(av_sb)

            # batched LN stats: -mean, variance, rsigma = exp(-0.5 ln(var+eps))
            negmu = sp.tile([D, npk], F32, name="negmu", tag="negmu")
            nc.vector.tensor_scalar_mul(negmu, avs, -1.0 / D)
            var = sp.tile([D, npk], F32, name="var", tag="var")
            nc.vector.tensor_scalar(var, sqs, 1.0 / D, 1e-5,
                                    op0=mybir.AluOpType.mult,
                                    op1=mybir.AluOpType.add)
            musq = sp.tile([D, npk], F32, name="musq", tag="musq")
            nc.vector.tensor_mul(musq, negmu, negmu)
            nc.vector.tensor_sub(var, var, musq)
            lnv = sp.tile([D, npk], F32, name="lnv", tag="lnv")
            nc.scalar.activation(lnv, var, mybir.ActivationFunctionType.Ln,
                                 bias=0.0, scale=1.0)
            rsig = sp.tile([D, npk], F32, name="rsig", tag="rsig")
            nc.scalar.activation(rsig, lnv, mybir.ActivationFunctionType.Exp,
                                 scale=-0.5)

            tT_sb = mp.tile([D, TT], BF16, name="tT_sb", tag="tT_sb")
            for p in range(npk):
                sl = slice(p * D, (p + 1) * D)
                av_sb = av_tiles[p]
                # z = (av - mu) * rsigma  (per-token scalars on partitions)
                z = sp.tile([D, D], F32, name="z", tag="z")
                nc.vector.tensor_scalar(z, av_sb, negmu[:, p:p + 1],
                                        rsig[:, p:p + 1],
                                        op0=mybir.AluOpType.add,
                                        op1=mybir.AluOpType.mult)
                if use_g:
                    nc.vector.tensor_mul(z, z, gb_sb[:, 0:D])
                if use_b:
                    nc.vector.tensor_add(z, z, gb_sb[:, D:2 * D])
                t_sb = sp.tile([D, D], BF16, name="t_sb", tag="t_sb")
                nc.vector.tensor_scalar_max(t_sb, z, 0.0)
                # transpose to feature-major for the out-projection
                tT_ps = tpps.tile([D, D], BF16, name="tT_ps", tag="tp")
                nc.tensor.transpose(tT_ps, t_sb, ident)
                nc.vector.tensor_copy(tT_sb[:, sl], tT_ps)

            # out-projection (feature-major) + max over the 16 neighbors
            oT_ps = bigps.tile([D, TT], F32, name="oT_ps", tag="big")
            nc.tensor.matmul(oT_ps, wo_sb, tT_sb, start=True, stop=True)
            nc.vector.tensor_reduce(
                stage[:, t * (TT // KN):(t + 1) * (TT // KN)],
                oT_ps.rearrange("p (g k) -> p g k", k=KN),
                axis=mybir.AxisListType.X, op=mybir.AluOpType.max)

        # + bo, downcast, transpose back to token-major f16, write out
        stage_bf = og.tile([D, npts], BF16, name="stage_bf")
        nc.vector.tensor_scalar_add(stage_bf, stage, bo_sb)
        for b in range(npts // D):
            sl = slice(b * D, (b + 1) * D)
            ot_ps = tpps.tile([D, D], BF16, name="ot_ps", tag="tp")
            nc.tensor.transpose(ot_ps, stage_bf[:, sl], ident)
            ot_sb = sp.tile([D, D], F16, name="ot_sb", tag="ot_sb")
            nc.vector.tensor_copy(ot_sb, ot_ps)
            nc.sync.dma_start(out=outT[sl, :], in_=ot_sb)

    nc.compile()
    return nc


class _Runner:
    """jit-once PJRT execution of the Bass program across 8 cores."""

    def __init__(self, nc, n_cores=8):
        import jax
        import jax.numpy as jnp
        from jax.experimental.shard_map import shard_map
        from jax.sharding import Mesh, NamedSharding, PartitionSpec
        from concourse.bass2jax import (_bass_exec_p, install_neuronx_cc_hook,
                                        partition_id_tensor)

        install_neuronx_cc_hook()
        self.jax = jax
        self.nc = nc
        assert getattr(nc, "dbg_addr", None) is None
        partition_name = (nc.partition_id_tensor.name
                          if nc.partition_id_tensor is not None else None)
        in_names, out_names, out_avals = [], [], []
        for alloc in nc.m.functions[0].allocations:
            if not isinstance(alloc, mybir.MemoryLocationSet):
                continue
            name = alloc.memorylocations[0].name
            if alloc.kind == "ExternalInput":
                if name != partition_name:
                    in_names.append(name)
            elif alloc.kind == "ExternalOutput":
                out_names.append(name)
                out_avals.append(jax.core.ShapedArray(
                    tuple(alloc.tensor_shape), mybir.dt.np(alloc.dtype)))
        self.in_names, self.out_names = in_names, out_names
        n_params, n_outs = len(in_names), len(out_names)
        all_names = in_names + out_names
        if partition_name is not None:
            all_names.append(partition_name)
        all_names = tuple(all_names)
        out_avals = tuple(out_avals)

        devices = jax.devices()[:n_cores]
        mesh = Mesh(np.asarray(devices), ("core",))
        self.sharding = NamedSharding(mesh, PartitionSpec("core"))

        def _body(*args):
            operands = list(args)
            if partition_name is not None:
                operands.append(partition_id_tensor())
            return tuple(_bass_exec_p.bind(
                *operands, out_avals=out_avals, in_names=all_names,
                out_names=tuple(out_names),
                lowering_input_output_aliases=(),
                sim_require_finite=True, sim_require_nnan=True, nc=nc))

        self.exec_fn = jax.jit(
            shard_map(_body, mesh=mesh,
                      in_specs=(PartitionSpec("core"),) * (n_params + n_outs),
                      out_specs=(PartitionSpec("core"),) * n_outs,
                      check_rep=False),
            donate_argnums=tuple(range(n_params, n_params + n_outs)),
            keep_unused=True)
        zinfo = [(tuple((n_cores * a.shape[0],) + a.shape[1:]), a.dtype)
                 for a in out_avals]
        self.zeros_fn = jax.jit(
            lambda: tuple(jnp.zeros(s, d) for s, d in zinfo),
            out_shardings=(self.sharding,) * n_outs)

    def run(self, host_arrays):
        """host_arrays: dict name -> global [8*shard0, ...] numpy array."""
        put = lambda a: self.jax.device_put(a, self.sharding)
        args = [put(host_arrays[n]) for n in self.in_names]
        outs = self.exec_fn(*args, *self.zeros_fn())
        return {n: np.asarray(o) for n, o in zip(self.out_names, outs)}


def kernel(x, context, Wck, bck, Wcb, bcb, Wq, bq, Wk, bk, Wv, bv,
           ln_g, ln_b, Wo, bo):
    """Full-input entry point: shards bs across 8 cores, returns full output."""
    global last_exec_time_ns
    t_start = time.perf_counter()
    x = np.asarray(x, dtype=np.float32)
    context = np.asarray(context, dtype=np.float32)
    f32 = lambda a: np.asarray(a, dtype=np.float32)
    Wck, bck, Wcb, bcb = f32(Wck), f32(bck), f32(Wcb), f32(bcb)
    Wq, bq, Wk, bk, Wv, bv = f32(Wq), f32(bq), f32(Wk), f32(bk), f32(Wv), f32(bv)
    ln_g, ln_b, Wo, bo = f32(ln_g), f32(ln_b), f32(Wo), f32(bo)

    bs, pn, kn, d = x.shape
    ntok = pn * kn
    npts = pn
    use_g = not np.allclose(ln_g, 1.0)
    use_b = np.any(ln_b != 0.0)

    key = (ntok, use_g, use_b)
    if key not in _cache:
        _cache[key] = _Runner(_build(ntok, use_g, use_b), n_cores=bs)
    runner = _cache[key]

    # fold the FiLM additive path (cb = ctx@Wcb + bcb) through q/k/v
    W2q, W2k, W2v = Wcb @ Wq, Wcb @ Wk, Wcb @ Wv
    bias_q = bq + bcb @ Wq
    bias_k = bk + bcb @ Wk
    bias_v = bv + bcb @ Wv
    gidx = np.arange(D) // KN
    Am = (gidx[None, :] == np.arange(8)[:, None]).astype(np.float32)
    Bm = np.where(Am > 0, 0.0, -65536.0).astype(np.float32)

    cw = CW_BASE + (2 * D if (use_g or use_b) else 0)
    wf = np.zeros((D, cw), np.float32)
    wf[:, _C_BCK] = bck
    wf[:, _C_BQ] = bias_q
    wf[:, _C_BK] = bias_k
    wf[:, _C_BO] = bo
    wf[0:CTX, _C_WCK:_C_WCK + D] = Wck
    wf[:, _C_WQ:_C_WQ + D] = Wq
    wf[:, _C_WK:_C_WK + D] = Wk
    wf[:, _C_WV:_C_WV + D] = Wv
    wf[:, _C_WO:_C_WO + D] = Wo
    wf[0:CTX, _C_W2Q:_C_W2Q + D] = W2q
    wf[0:CTX, _C_W2K:_C_W2K + D] = W2k
    wf[0:CTX, _C_W2V:_C_W2V + D] = W2v
    wf[0:8, _C_AM:_C_AM + D] = Am
    wf[0:8, _C_BM:_C_BM + D] = Bm
    wf[0:1, _C_BVR:_C_BVR + D] = bias_v
    if use_g or use_b:
        wf[:, _C_GB:_C_GB + D] = np.broadcast_to(ln_g[:, None], (D, D)).T
        wf[:, _C_GB + D:_C_GB + 2 * D] = np.broadcast_to(ln_b[:, None], (D, D)).T

    host_arrays = {
        "xTok": x.reshape(bs * ntok, d).astype(BF),
        "cTok": context.reshape(bs * ntok, CTX).astype(BF),
        "wf": np.tile(wf, (bs, 1)),
    }
    res = runner.run(host_arrays)
    out = res["outT"].reshape(bs, npts, d).astype(np.float32)
    last_exec_time_ns = int((time.perf_counter() - t_start) * 1e9)
    return out
